# revision 1
# baseline (speedup 1.0000x reference)
"""Causal Performer attention on 8 trn2 NeuronCores.

Sharding: core c handles batch b = c // 4 and head-group hg = c % 4
(3 of the 12 heads). Each core:
  1. computes the qkv projection for its 3 heads (576 of 2304 rows),
  2. runs the causal linear-attention scan in chunked form (the
     (T,F,D) cumsum tensor is never materialized: intra-chunk masked
     (K'Q') scores plus an inter-chunk (F,D) running state),
  3. computes a partial output projection over its 192 channels.
The host sums the 4 partial (C,T) projections per batch and
transposes back to (B,T,C).

All device matmuls keep the contraction dim on partitions, so the
host pre-transposes x and the weight slices (pure layout work).

Math notes:
  - The 1/sqrt(F) factor on q' and k' cancels in numerator/denominator;
    it is dropped and EPS is scaled by F to keep the result exact.
  - q'^T = exp(P^T q - 0.5|q|^2) is produced directly in [f, t] layout
    by a single matmul with stacked stationary [proj; -0.5*ones] and
    stacked moving [q^T; (q^2)^T]: the -0.5*ones block replicates
    -0.5|q[t]|^2 across all f partitions, so the ACT pass is a pure Exp.
  - K' in natural [t, f] layout and V in natural [t, d] layout are each
    produced by their own matmul orientation (no PE transposes).
  - The [V | 1] extended operand makes each scan matmul produce the
    numerator and denominator together (extra column).

Scheduling structure (Tile executes each engine's stream in emission
order, so emission order is the schedule):
  - One PSUM pool with four static tags (2 banks each) so no pool-scope
    barrier ever serializes phases.
  - The qkv matmul streams k-tiles as the DMAs land; (q_h, k_h) share an
    M-tile so head h's whole pipeline (squares -> q'/k' -> K'nat ->
    scan) starts as soon as its slice is evicted.
  - The output projection runs nh-outer so the first half of the output
    stores while the scan's second half still runs.
"""

import numpy as np

import concourse.bacc as bacc
import concourse.bass as bass
import concourse.mybir as mybir
from concourse import tile
from concourse.bass_utils import run_bass_kernel_spmd

B, T, C = 2, 1024, 768
H, D, F = 12, 64, 64
HPC = 3  # heads per core
S = 128  # scan chunk length
NCH = T // S  # 8 chunks
CP = HPC * D  # 192 channels per core
N_CORES = 8
KT = C // 128  # 6 contraction tiles for the qkv matmul
EPS_SCALED = float(F) * 1e-6  # compensates dropping 1/sqrt(F) on q', k'

FP32 = mybir.dt.float32
F32R = mybir.dt.float32r
EXP = mybir.ActivationFunctionType.Exp
COPY = mybir.ActivationFunctionType.Copy

DT_IN = F32R  # xt, wt, wpt dram+sbuf dtype (verifier accepts DMA'd f32r)
DT_STK = F32R  # stacked [qT; q^2T] tiles


DMA_ONLY = False
FRONT_ONLY = False


def build_program(n_iters=1):
    nc = bacc.Bacc(
        "TRN2", target_bir_lowering=False, debug=False, num_devices=N_CORES
    )
    xt = nc.dram_tensor("xt", [C, T], DT_IN, kind="ExternalInput").ap()
    wt = nc.dram_tensor("wt", [C, 3 * CP], DT_IN, kind="ExternalInput").ap()
    wpt = nc.dram_tensor("wpt", [256, C], DT_IN, kind="ExternalInput").ap()
    projext = nc.dram_tensor("projext", [2 * D, F], DT_IN, kind="ExternalInput").ap()
    mask = nc.dram_tensor("mask", [S, S], FP32, kind="ExternalInput").ap()
    ident = nc.dram_tensor("ident", [128, 128], FP32, kind="ExternalInput").ap()
    yt = nc.dram_tensor("yt", [C, T], FP32, kind="ExternalOutput").ap()

    from contextlib import ExitStack

    with tile.TileContext(nc) as tc:
        for _ in range(n_iters):
            with ExitStack() as ctx:
                _body(ctx, tc, xt, wt, wpt, projext, mask, ident, yt)
    nc.compile()
    return nc


def _body(ctx, tc, xt, wt, wpt, projext, mask, ident, yt):
    nc = tc.nc

    const = ctx.enter_context(tc.tile_pool(name="const", bufs=1))
    mask_s = const.tile([S, S], FP32, name="mask", tag="mask")
    nc.sync.dma_start(mask_s[:], mask)
    ident_s = const.tile([128, 128], FP32, name="ident", tag="ident")
    nc.sync.dma_start(ident_s[:], ident)
    projext_s = const.tile([2 * D, F], DT_IN, name="projext", tag="projext")
    nc.sync.dma_start(projext_s[:], projext)

    # inputs split across the two HWDGE queues (sync=SP, scalar=ACT);
    # wpt last (only needed by the output projection)
    big = ctx.enter_context(tc.tile_pool(name="big", bufs=1))
    xt_s = []
    wt_s = []
    for k in range(KT):
        tx = big.tile([128, T], DT_IN, name=f"xt{k}", tag=f"xt{k}")
        nc.sync.dma_start(tx[:], xt[k * 128 : (k + 1) * 128, :])
        xt_s.append(tx)
        tw = big.tile([128, 3 * CP], DT_IN, name=f"wt{k}", tag=f"wt{k}")
        nc.scalar.dma_start(tw[:], wt[k * 128 : (k + 1) * 128, :])
        wt_s.append(tw)
    wpt_a = big.tile([128, C], DT_IN, name="wpt_a", tag="wpt_a")
    nc.scalar.dma_start(wpt_a[:], wpt[0:128, :])
    wpt_b = big.tile([128, C], DT_IN, name="wpt_b", tag="wpt_b")
    nc.scalar.dma_start(wpt_b[:], wpt[128:256, :])

    if DMA_ONLY:
        ysb0 = ctx.enter_context(tc.tile_pool(name="y0", bufs=2))
        for ot in range(C // 128):
            yo = ysb0.tile([128, T], FP32, name="yo0", tag="yo0")
            nc.vector.tensor_copy(yo[:, 0:128], xt_s[ot][:, 0:128])
            eng = nc.sync if ot % 2 == 0 else nc.scalar
            eng.dma_start(yt[ot * 128 : (ot + 1) * 128, :], yo[:])
        return

    # single PSUM pool, static tags: A (projections), B (vdir/numden),
    # C (scores/transposes), St (scan state) -- 2 banks each
    psum = ctx.enter_context(tc.tile_pool(name="psum", bufs=2, space="PSUM"))

    # persistent SBUF tensors
    stk = {}  # (kind, h): rows 0:64 = (q|k)^T head h, rows 64:128 squared
    qpT = {}  # (kind, h): [F, T] Q'^T / K'^T
    for kind in "qk":
        for h in range(HPC):
            stk[(kind, h)] = big.tile(
                [128, T], DT_STK, name=f"stk{kind}{h}", tag=f"stk{kind}{h}"
            )
            qpT[(kind, h)] = big.tile(
                [F, T], DT_IN, name=f"qpT{kind}{h}", tag=f"qpT{kind}{h}"
            )
    vext = {}  # (h, chunk) -> [S, D+1], col D = 1.0
    for h in range(HPC):
        for i in range(NCH):
            vext[(h, i)] = big.tile(
                [S, D + 1], FP32, name=f"vx{h}_{i}", tag=f"vx{h}_{i}"
            )
    knat = {}  # (h, pair) -> [128, 128]: chunks 2p | 2p+1 in col halves
    for h in range(HPC):
        for p in range(NCH // 2):
            knat[(h, p)] = big.tile(
                [S, 128], FP32, name=f"kn{h}_{p}", tag=f"kn{h}_{p}"
            )
    outT01 = big.tile([128, T], DT_IN, name="outT01", tag="outT01")
    outT2 = big.tile([128, T], DT_IN, name="outT2", tag="outT2")
    nc.gpsimd.memset(outT2[D:128, :].bitcast(FP32), 0.0)

    vdir_chunks_done = [0]

    def emit_vdir(upto):
        # V in natural [t, d] layout via its own matmul; wt cols 320:576
        # give [64 junk | 192 v] per t-chunk
        while vdir_chunks_done[0] < upto:
            i = vdir_chunks_done[0]
            pt = psum.tile([S, 256], FP32, name="vdp", tag="B")
            for k in range(KT):
                nc.tensor.matmul(
                    pt[:],
                    xt_s[k][:, i * S : (i + 1) * S],
                    wt_s[k][:, 320:576],
                    start=(k == 0),
                    stop=(k == KT - 1),
                )
            vnat = big.tile([S, CP], FP32, name=f"vnat{i}", tag=f"vnat{i}")
            nc.vector.tensor_copy(vnat[:], pt[:, 64:256])
            for h in range(HPC):
                ve = vext[(h, i)]
                nc.gpsimd.tensor_copy(ve[:, 0:D], vnat[:, h * D : (h + 1) * D])
                nc.gpsimd.memset(ve[:, D : D + 1], 1.0)
            vdir_chunks_done[0] += 1

    # --- per-head front-end: qkv M-tile (q_h | k_h), squares, q'/k', K'nat ---
    for h in range(HPC):
        # qkv: k-streamed, both t-halves live
        pts = [
            psum.tile([128, 512], FP32, name=f"qkvp{h}{nh}", tag="A")
            for nh in range(2)
        ]
        for k in range(KT):
            lhs = wt_s[k][:, h * 128 : (h + 1) * 128]
            for nh in range(2):
                nc.tensor.matmul(
                    pts[nh][:],
                    lhs,
                    xt_s[k][:, nh * 512 : (nh + 1) * 512],
                    start=(k == 0),
                    stop=(k == KT - 1),
                )
        for nh in range(2):
            dst_cols = slice(nh * 512, (nh + 1) * 512)
            for half, kind in enumerate("qk"):
                src = pts[nh][half * 64 : (half + 1) * 64, :]
                st = stk[(kind, h)]
                if half == 0:
                    nc.scalar.activation(st[0:64, dst_cols], src, COPY)
                else:
                    nc.vector.tensor_copy(st[0:64, dst_cols], src)
                nc.vector.tensor_mul(
                    st[64:128, dst_cols],
                    st[0:64, dst_cols],
                    st[0:64, dst_cols],
                )

        if h == 0:
            emit_vdir(2)

        # q'/k' in [f, t] layout: one matmul + pure Exp
        for kind in "qk":
            for nh in range(2):
                cols = slice(nh * 512, (nh + 1) * 512)
                pt = psum.tile([F, 512], FP32, name="pp", tag="A")
                nc.tensor.matmul(
                    pt[:], projext_s[:], stk[(kind, h)][:, cols],
                    start=True, stop=True,
                )
                nc.scalar.activation(qpT[(kind, h)][:, cols], pt[:], EXP)

        # K' natural [t, f], chunk-paired psum -> one Exp per pair
        for p in range(NCH // 2):
            pt = psum.tile([S, 128], FP32, name="knp", tag="C")
            for half in range(2):
                i = 2 * p + half
                nc.tensor.matmul(
                    pt[:, half * 64 : (half + 1) * 64],
                    stk[("k", h)][:, i * S : (i + 1) * S].bitcast(FP32),
                    projext_s[:].bitcast(FP32),
                    start=True,
                    stop=True,
                )
            nc.scalar.activation(knat[(h, p)][:], pt[:], EXP)

        emit_vdir(2 * (h + 1) + 2)

    emit_vdir(NCH)

    if FRONT_ONLY:
        for h in range(HPC):
            eng = nc.sync if h % 2 == 0 else nc.scalar
            eng.dma_start(
                yt[h * 128 : h * 128 + 64, :], qpT[("q", h)][:].bitcast(FP32)
            )
            eng.dma_start(
                yt[(3 + h) * 128 : (3 + h) * 128 + 64, :],
                qpT[("k", h)][:].bitcast(FP32),
            )
        return

    # --- scan: chunk pairs (2p, 2p+1); one f32r [128,256] score matmul
    # per pair covers the masked own-block of 2p plus the full cross block
    # (2p -> 2p+1); the state advances once per pair ---
    sb = ctx.enter_context(tc.tile_pool(name="scan_sb", bufs=6))
    ysb = ctx.enter_context(tc.tile_pool(name="y_sb", bufs=6))
    emit_yproj = _yproj_maker(nc, psum, ysb, wpt_a, wpt_b, outT01, outT2, yt)
    och_pair = {}
    och2 = {}
    for i in range(NCH):
        och_pair[i] = sb.tile(
            [S, 128], FP32, name=f"ochp{i}", tag="ochp", bufs=NCH + 1
        )
        och2[i] = sb.tile([S, D], FP32, name=f"och2_{i}", tag="och2", bufs=NCH + 1)

    def division(h, i, nd):
        dinv = sb.tile([S, 1], FP32, name="dinv", tag="dinv")
        nc.vector.tensor_scalar_add(dinv[:], nd[:, D : D + 1], EPS_SCALED)
        nc.vector.reciprocal(dinv[:], dinv[:])
        och = och_pair[i][:, h * D : (h + 1) * D] if h < 2 else och2[i][:]
        if (h + i) % 2 == 0:
            nc.scalar.activation(och, nd[:, 0:D], COPY, scale=dinv[:])
        else:
            nc.vector.tensor_scalar_mul(och, nd[:, 0:D], dinv[:])

    for h in range(HPC):
        qTh = qpT[("q", h)]
        kTh = qpT[("k", h)]
        state_ps = psum.tile([F, D + 1], FP32, name=f"state{h}", tag="St")

        for p in range(NCH // 2):
            i0, i1 = 2 * p, 2 * p + 1
            c0 = slice(i0 * S, (i0 + 1) * S)
            c1 = slice(i1 * S, (i1 + 1) * S)
            cpair = slice(i0 * S, (i0 + 2) * S)

            # packed scores: K'[i0] x Q'[i0|i1] (f32r) then own block of i1
            stp = psum.tile([S, 384], FP32, name="stp", tag="C")
            nc.tensor.matmul(
                stp[:, 0:256], kTh[:, c0], qTh[:, cpair], start=True, stop=True
            )
            nc.tensor.matmul(
                stp[:, 256:384], kTh[:, c1].bitcast(FP32),
                qTh[:, c1].bitcast(FP32), start=True, stop=True,
            )
            stm0 = sb.tile([S, S], FP32, name="stm0", tag="stm")
            nc.vector.tensor_mul(stm0[:], stp[:, 0:S], mask_s[:])
            stx = sb.tile([S, S], FP32, name="stx", tag="stx")
            if h == 0:
                nc.vector.tensor_copy(stx[:], stp[:, S : 2 * S])
            else:
                nc.scalar.activation(stx[:], stp[:, S : 2 * S], COPY)
            stm1 = sb.tile([S, S], FP32, name="stm1", tag="stm")
            nc.vector.tensor_mul(stm1[:], stp[:, 256:384], mask_s[:])

            if p > 0:
                ssb = sb.tile([F, D + 1], FP32, name="ssb", tag="ssb")
                if h == 0:
                    nc.vector.tensor_copy(ssb[:], state_ps[:])
                else:
                    nc.scalar.activation(ssb[:], state_ps[:], COPY)

            # packed numden, sequential groups (a bank's zero region is
            # lazily zeroed on write, so close group 0 before group 1 opens)
            ndp = psum.tile([S, 2 * (D + 1)], FP32, name="ndp", tag="B")
            nd0 = ndp[:, 0 : D + 1]
            nd1 = ndp[:, D + 1 : 2 * (D + 1)]
            nc.tensor.matmul(
                nd0, stm0[:], vext[(h, i0)][:], start=True, stop=(p == 0)
            )
            if p > 0:
                nc.tensor.matmul(
                    nd0, qTh[:, c0].bitcast(FP32), ssb[:],
                    start=False, stop=True,
                )
            nc.tensor.matmul(
                nd1, stm1[:], vext[(h, i1)][:], start=True, stop=False
            )
            nc.tensor.matmul(
                nd1, stx[:], vext[(h, i0)][:], start=False, stop=(p == 0)
            )
            if p > 0:
                nc.tensor.matmul(
                    nd1, qTh[:, c1].bitcast(FP32), ssb[:],
                    start=False, stop=True,
                )
            division(h, i0, nd0)
            division(h, i1, nd1)

            # state += K'^T [V | 1] for both chunks of the pair
            for i in (i0, i1):
                nc.tensor.matmul(
                    state_ps[:],
                    knat[(h, i // 2)][:, (i % 2) * 64 : (i % 2 + 1) * 64],
                    vext[(h, i)][:],
                    start=(i == 0),
                    stop=True,
                    skip_group_check=True,
                )

            # transposes as soon as a pair's outputs exist (tag A is idle
            # during the scan)
            if h == 1:
                for i in (i0, i1):
                    cols = slice(i * S, (i + 1) * S)
                    tp = psum.tile([128, S], FP32, name="tp", tag="A")
                    nc.tensor.transpose(tp[:], och_pair[i][:], ident_s[:])
                    nc.vector.tensor_copy(outT01[:, cols], tp[:])
            elif h == 2:
                for i in (i0, i1):
                    cols = slice(i * S, (i + 1) * S)
                    tp2 = psum.tile([D, S], FP32, name="tp2", tag="A")
                    nc.tensor.transpose(tp2[:], och2[i][:], ident_s[:])
                    nc.scalar.activation(outT2[0:D, cols], tp2[:], COPY)
                if p % 2 == 1:
                    emit_yproj(p // 2)

    # --- partial output projection yt = wpt.T @ outT (emitted inside the
    # h2 scan via emit_yproj) ---


def _yproj_maker(nc, psum, ysb, wpt_a, wpt_b, outT01, outT2, yt):
    def emit_yproj(nh):
        cols = slice(nh * 512, (nh + 1) * 512)
        for ot in range(C // 128):
            ypt = psum.tile([128, 512], FP32, name="ypt", tag="A")
            nc.tensor.matmul(
                ypt[:],
                wpt_a[:, ot * 128 : (ot + 1) * 128],
                outT01[:, cols],
                start=True,
                stop=False,
            )
            nc.tensor.matmul(
                ypt[:],
                wpt_b[:, ot * 128 : (ot + 1) * 128],
                outT2[:, cols],
                start=False,
                stop=True,
            )
            yo = ysb.tile([128, 512], FP32, name="yo", tag="yo")
            if ot % 2 == 0:
                nc.vector.tensor_copy(yo[:], ypt[:])
            else:
                nc.scalar.activation(yo[:], ypt[:], COPY)
            dma_eng = nc.sync if ot % 2 == 0 else nc.scalar
            dma_eng.dma_start(yt[ot * 128 : (ot + 1) * 128, cols], yo[:])

    return emit_yproj


_PROGRAM = None


def _get_program():
    global _PROGRAM
    if _PROGRAM is None:
        _PROGRAM = build_program()
    return _PROGRAM


def make_core_inputs(x, W_attn, W_proj, proj, core):
    b, hg = divmod(core, 4)
    heads = list(range(HPC * hg, HPC * (hg + 1)))
    rows = []
    for h in heads:  # (q_h | k_h) pairs, then the v block
        rows.extend(range(h * D, (h + 1) * D))
        rows.extend(range(C + h * D, C + (h + 1) * D))
    for h in heads:
        rows.extend(range(2 * C + h * D, 2 * C + (h + 1) * D))
    projext = np.concatenate(
        [proj.astype(np.float32), np.full((D, F), -0.5, np.float32)], axis=0
    )
    return {
        "xt": np.ascontiguousarray(x[b].T),
        "wt": np.ascontiguousarray(W_attn[rows, :].T),
        "wpt": np.concatenate(
            [
                np.ascontiguousarray(W_proj[:, CP * hg : CP * (hg + 1)].T),
                np.zeros((256 - CP, C), np.float32),
            ],
            axis=0,
        ),
        "projext": projext,
        "mask": np.triu(np.ones((S, S), np.float32)),
        "ident": np.eye(128, dtype=np.float32),
    }


def kernel(x, W_attn, W_proj, proj):
    nc = _get_program()
    in_maps = [
        make_core_inputs(x, W_attn, W_proj, proj, core) for core in range(N_CORES)
    ]
    res = run_bass_kernel_spmd(nc, in_maps, list(range(N_CORES)))
    out = np.empty((B, T, C), np.float32)
    for b in range(B):
        acc = res.results[4 * b]["yt"].astype(np.float32).copy()
        for g in range(1, 4):
            acc += res.results[4 * b + g]["yt"]
        out[b] = acc.T
    return out



# revision 5
# speedup vs baseline: 2.3848x; 2.3848x over previous
"""Causal Performer attention on 8 trn2 NeuronCores — bf16 pipeline.

Sharding: core c handles batch b = c // 4 and head-group hg = c % 4
(3 of the 12 heads). Each core:
  1. computes the qkv projection for its 3 heads (576 of 2304 rows),
  2. runs the causal linear-attention scan in chunked form (the
     (T,F,D) cumsum tensor is never materialized: intra-chunk masked
     (K'Q') scores plus an inter-chunk (F,D) running state),
  3. computes a partial output projection over its 192 channels.
The host sums the 4 partial (C,T) projections per batch and
transposes back to (B,T,C).

Numerics: everything SBUF-resident is bf16 (matmuls run 1 cycle/row at
any free size; DVE gets 2-4x on 2-byte SBUF-only ops; HBM traffic is
halved); PSUM accumulation stays fp32 and the final output is stored
fp32. The 1/sqrt(F) on q', k' cancels in num/den; EPS is scaled by F.

Layouts:
  - stk_{q,k} per head: rows 0:64 raw (q|k), rows 64:128 squared —
    squares computed SBUF->SBUF on DVE at 4x after a single psum copy.
  - q'^T/k'^T per head [F, T], produced by two partition-split prime
    matmuls into one [128,512] psum + two Exps (base-partition rules
    forbid mixing halves of one tile as matmul operands).
  - K' natural [t, f] obtained by PE-transposing k'^T (no second
    exp); packed 4 chunks per [S, 256] tile.
  - [V | 1] extended operand gives numerator+denominator in one
    matmul column block; the ones column is memset exactly once.
  - Scores for a chunk pair sit in one [S, 384] psum ([own0 | cross |
    own1]); a single [S,384] DVE multiply with the [tri|ones|tri]
    mask replaces three ops.
  - Division uses a [S,2] strided-AP extract of both denominators.

Scheduling: single PSUM pool with static tags (A projections/
transposes, B vdir/numden, C scores/knat, St scan state); emission
order interleaves vdir into the per-head front-end and the output
projection into head 2's scan, as in the fp32 baseline.
"""

import numpy as np
import ml_dtypes

import concourse.bacc as bacc
import concourse.bass as bass
import concourse.mybir as mybir
from concourse import tile
from concourse.bass_utils import run_bass_kernel_spmd

B, T, C = 2, 1024, 768
H, D, F = 12, 64, 64
HPC = 3  # heads per core
S = 128  # scan chunk length
NCH = T // S  # 8 chunks
CP = HPC * D  # 192 channels per core
N_CORES = 8
KT = C // 128  # 6 contraction tiles for the qkv matmul
EPS_SCALED = float(F) * 1e-6  # compensates dropping 1/sqrt(F) on q', k'

FP32 = mybir.dt.float32
BF16 = mybir.dt.bfloat16
EXP = mybir.ActivationFunctionType.Exp
COPY = mybir.ActivationFunctionType.Copy
SQUARE = mybir.ActivationFunctionType.Square


def build_program(n_iters=1):
    nc = bacc.Bacc(
        "TRN2", target_bir_lowering=False, debug=False, num_devices=N_CORES
    )
    xt = nc.dram_tensor("xt", [C, T], BF16, kind="ExternalInput").ap()
    wt = nc.dram_tensor("wt", [C, 3 * CP], BF16, kind="ExternalInput").ap()
    wpt = nc.dram_tensor("wpt", [CP, C], BF16, kind="ExternalInput").ap()
    projext = nc.dram_tensor("projext", [2 * D, F], BF16, kind="ExternalInput").ap()
    mask3 = nc.dram_tensor("mask3", [S, 3 * S], BF16, kind="ExternalInput").ap()
    ident = nc.dram_tensor("ident", [128, 128], BF16, kind="ExternalInput").ap()
    yt = nc.dram_tensor("yt", [C, T], FP32, kind="ExternalOutput").ap()

    from contextlib import ExitStack

    with tile.TileContext(nc) as tc:
        for _ in range(n_iters):
            with ExitStack() as ctx:
                _body(ctx, tc, xt, wt, wpt, projext, mask3, ident, yt)
    nc.compile()
    return nc


def _body(ctx, tc, xt, wt, wpt, projext, mask3, ident, yt):
    nc = tc.nc

    const = ctx.enter_context(tc.tile_pool(name="const", bufs=1))
    mask_s = const.tile([S, 3 * S], BF16, name="mask3", tag="mask3")
    nc.sync.dma_start(mask_s[:], mask3)
    ident_s = const.tile([128, 128], BF16, name="ident", tag="ident")
    nc.sync.dma_start(ident_s[:], ident)
    projext_s = const.tile([2 * D, F], BF16, name="projext", tag="projext")
    nc.sync.dma_start(projext_s[:], projext)

    # inputs split across the two HWDGE queues (sync=SP, scalar=ACT);
    # wpt last (only needed by the output projection)
    big = ctx.enter_context(tc.tile_pool(name="big", bufs=1))
    xt_s = []
    wt_s = []
    for k in range(KT):
        tx = big.tile([128, T], BF16, name=f"xt{k}", tag=f"xt{k}")
        nc.sync.dma_start(tx[:], xt[k * 128 : (k + 1) * 128, :])
        xt_s.append(tx)
        tw = big.tile([128, 3 * CP], BF16, name=f"wt{k}", tag=f"wt{k}")
        nc.scalar.dma_start(tw[:], wt[k * 128 : (k + 1) * 128, :])
        wt_s.append(tw)
    wpt_a = big.tile([128, C], BF16, name="wpt_a", tag="wpt_a")
    nc.scalar.dma_start(wpt_a[:], wpt[0:128, :])
    wpt_b = big.tile([CP - 128, C], BF16, name="wpt_b", tag="wpt_b")
    nc.scalar.dma_start(wpt_b[:], wpt[128:CP, :])

    # single PSUM pool, static tags: A (projections/transposes/yproj),
    # B (vdir/numden), C (scores/knat), St (scan state) -- 2 banks each
    psum = ctx.enter_context(tc.tile_pool(name="psum", bufs=2, space="PSUM"))

    # persistent SBUF tensors
    stk = {}  # (kind, h): rows 0:64 = (q|k) head h, rows 64:128 squared
    qpT = {}  # h: [F, T] q'^T
    kpT = {}  # h: [F, T] k'^T
    for h in range(HPC):
        for kind in "qk":
            stk[(kind, h)] = big.tile(
                [128, T], BF16, name=f"stk{kind}{h}", tag=f"stk{kind}{h}"
            )
        qpT[h] = big.tile([F, T], BF16, name=f"qpT{h}", tag=f"qpT{h}")
        kpT[h] = big.tile([F, T], BF16, name=f"kpT{h}", tag=f"kpT{h}")
    vext = {}  # (h, chunk) -> [S, D+1], col D = 1.0 (memset once)
    for h in range(HPC):
        for i in range(NCH):
            vext[(h, i)] = big.tile(
                [S, D + 1], BF16, name=f"vx{h}_{i}", tag=f"vx{h}_{i}"
            )
            nc.gpsimd.memset(vext[(h, i)][:, D : D + 1], 1.0)
    knatg = {}  # (h, g) -> [S, 256]: chunks 4g..4g+3 in col quarters
    for h in range(HPC):
        for g in range(2):
            knatg[(h, g)] = big.tile(
                [S, 256], BF16, name=f"kn{h}_{g}", tag=f"kn{h}_{g}"
            )
    outT01 = big.tile([128, T], BF16, name="outT01", tag="outT01")
    outT2 = big.tile([D, T], BF16, name="outT2", tag="outT2")

    vdir_chunks_done = [0]
    vnat_sb = ctx.enter_context(tc.tile_pool(name="vnat_sb", bufs=3))

    def emit_vdir(upto):
        # V in natural [t, d] layout via its own matmul (wt cols 384:576)
        while vdir_chunks_done[0] < upto:
            i = vdir_chunks_done[0]
            pt = psum.tile([S, CP], FP32, name="vdp", tag="B")
            for k in range(KT):
                nc.tensor.matmul(
                    pt[:],
                    xt_s[k][:, i * S : (i + 1) * S],
                    wt_s[k][:, 2 * CP : 3 * CP],
                    start=(k == 0),
                    stop=(k == KT - 1),
                )
            vnat = vnat_sb.tile([S, CP], BF16, name="vnat", tag="vnat")
            if i % 2 == 0:
                nc.vector.tensor_copy(vnat[:], pt[:])
            else:
                nc.scalar.activation(vnat[:], pt[:], COPY)
            for h in range(HPC):
                nc.gpsimd.tensor_copy(
                    vext[(h, i)][:, 0:D], vnat[:, h * D : (h + 1) * D]
                )
            vdir_chunks_done[0] += 1

    # --- per-head front-end: qkv M-tile (q_h | k_h), raw copies, SBUF
    # squares, merged q'/k' prime + one Exp, K'nat via PE transpose ---
    for h in range(HPC):
        pts = [
            psum.tile([128, 512], FP32, name=f"qkvp{h}{nh}", tag="A")
            for nh in range(2)
        ]
        for k in range(KT):
            lhs = wt_s[k][:, h * 128 : (h + 1) * 128]
            for nh in range(2):
                nc.tensor.matmul(
                    pts[nh][:],
                    lhs,
                    xt_s[k][:, nh * 512 : (nh + 1) * 512],
                    start=(k == 0),
                    stop=(k == KT - 1),
                )
        for nh in range(2):
            cols = slice(nh * 512, (nh + 1) * 512)
            sq, sk = stk[("q", h)], stk[("k", h)]
            nc.scalar.activation(sq[0:64, cols], pts[nh][0:64, :], COPY)
            nc.vector.tensor_copy(sk[0:64, cols], pts[nh][64:128, :])
            # squares SBUF->SBUF (bf16, 4x DVE)
            nc.vector.tensor_mul(sq[64:128, cols], sq[0:64, cols], sq[0:64, cols])
            nc.vector.tensor_mul(sk[64:128, cols], sk[0:64, cols], sk[0:64, cols])

        if h == 0:
            emit_vdir(2)

        # q'^T/k'^T: two partition-split matmuls into one psum, two Exps
        for nh in range(2):
            cols = slice(nh * 512, (nh + 1) * 512)
            pp = psum.tile([128, 512], FP32, name="pp", tag="A")
            nc.tensor.matmul(
                pp[0:64, :], projext_s[:], stk[("q", h)][:, cols],
                start=True, stop=True,
            )
            nc.tensor.matmul(
                pp[64:128, :], projext_s[:], stk[("k", h)][:, cols],
                start=True, stop=True,
            )
            nc.scalar.activation(qpT[h][:, cols], pp[0:64, :], EXP)
            nc.scalar.activation(kpT[h][:, cols], pp[64:128, :], EXP)

        # K' natural [t, f] by transposing k'^T; 4 chunks per psum tile
        for g in range(2):
            tp = psum.tile([128, 256], BF16, name="knp", tag="C")
            for j in range(4):
                i = 4 * g + j
                nc.tensor.transpose(
                    tp[:, j * 64 : (j + 1) * 64],
                    kpT[h][:, i * S : (i + 1) * S],
                    ident_s[0:64, 0:64],
                )
            if (h + g) % 2 == 0:
                nc.vector.tensor_copy(knatg[(h, g)][:], tp[:])
            else:
                nc.scalar.activation(knatg[(h, g)][:], tp[:], COPY)

        emit_vdir(2 * (h + 1) + 2)

    emit_vdir(NCH)

    # --- scan: chunk pairs (2p, 2p+1); scores land in one [S,384] psum
    # ([own0 | cross | own1]); one masked multiply; state advances once
    # per pair with an explicit cross block covering i0 -> i1 ---
    sb = ctx.enter_context(tc.tile_pool(name="scan_sb", bufs=6))
    ysb = ctx.enter_context(tc.tile_pool(name="y_sb", bufs=6))
    emit_yproj = _yproj_maker(nc, psum, ysb, wpt_a, wpt_b, outT01, outT2, yt)
    och_pair = {}
    och2 = {}
    for i in range(NCH):
        och_pair[i] = sb.tile(
            [S, 128], BF16, name=f"ochp{i}", tag="ochp", bufs=NCH + 1
        )
        och2[i] = sb.tile([S, D], BF16, name=f"och2_{i}", tag="och2", bufs=NCH + 1)

    for h in range(HPC):
        state_ps = psum.tile([F, D + 1], FP32, name=f"state{h}", tag="St")

        for p in range(NCH // 2):
            i0, i1 = 2 * p, 2 * p + 1
            c0 = slice(i0 * S, (i0 + 1) * S)
            c1 = slice(i1 * S, (i1 + 1) * S)
            cpair = slice(i0 * S, (i0 + 2) * S)

            # packed scores: [K'0 x Q'(0|1) | K'1 x Q'1]
            stp = psum.tile([S, 384], FP32, name="stp", tag="C")
            nc.tensor.matmul(
                stp[:, 0:256], kpT[h][:, c0], qpT[h][:, cpair],
                start=True, stop=True,
            )
            nc.tensor.matmul(
                stp[:, 256:384], kpT[h][:, c1], qpT[h][:, c1],
                start=True, stop=True,
            )
            stm = sb.tile([S, 384], BF16, name="stm", tag="stm")
            nc.vector.tensor_mul(stm[:], stp[:], mask_s[:])

            if p > 0:
                ssb = sb.tile([F, D + 1], BF16, name="ssb", tag="ssb")
                if (h + p) % 2 == 0:
                    nc.vector.tensor_copy(ssb[:], state_ps[:])
                else:
                    nc.scalar.activation(ssb[:], state_ps[:], COPY)

            # packed numden, sequential groups (a bank's zero region is
            # lazily zeroed on write, so close group 0 before group 1 opens)
            ndp = psum.tile([S, 2 * (D + 1)], FP32, name="ndp", tag="B")
            nd0 = ndp[:, 0 : D + 1]
            nd1 = ndp[:, D + 1 : 2 * (D + 1)]
            nc.tensor.matmul(
                nd0, stm[:, 0:128], vext[(h, i0)][:], start=True, stop=(p == 0)
            )
            if p > 0:
                nc.tensor.matmul(
                    nd0, qpT[h][:, c0], ssb[:], start=False, stop=True
                )
            nc.tensor.matmul(
                nd1, stm[:, 256:384], vext[(h, i1)][:], start=True, stop=False
            )
            nc.tensor.matmul(
                nd1, stm[:, 128:256], vext[(h, i0)][:], start=False, stop=(p == 0)
            )
            if p > 0:
                nc.tensor.matmul(
                    nd1, qpT[h][:, c1], ssb[:], start=False, stop=True
                )

            # packed division: [S,2] strided denominator extract
            dinv = sb.tile([S, 2], FP32, name="dinv", tag="dinv")
            nc.scalar.activation(
                dinv[:], ndp[:, D : 2 * (D + 1) : D + 1], COPY, bias=EPS_SCALED
            )
            nc.vector.reciprocal(dinv[:], dinv[:])
            for idx, i in enumerate((i0, i1)):
                och = (
                    och_pair[i][:, h * D : (h + 1) * D] if h < 2 else och2[i][:]
                )
                src = ndp[:, idx * (D + 1) : idx * (D + 1) + D]
                dv = dinv[:, idx : idx + 1]
                if (h + i) % 2 == 0:
                    nc.scalar.activation(och, src, COPY, scale=dv)
                else:
                    nc.vector.tensor_scalar_mul(och, src, dv)

            # state += K'^T [V | 1] for both chunks of the pair
            for i in (i0, i1):
                nc.tensor.matmul(
                    state_ps[:],
                    knatg[(h, i // 4)][:, (i % 4) * 64 : (i % 4 + 1) * 64],
                    vext[(h, i)][:],
                    start=(i == 0),
                    stop=True,
                    skip_group_check=True,
                )

            # transposes as soon as a pair's outputs exist (tag A is idle
            # during the scan)
            if h == 1:
                tp = psum.tile([128, 256], BF16, name="tp", tag="A")
                nc.tensor.transpose(tp[:, 0:128], och_pair[i0][:], ident_s[:])
                nc.tensor.transpose(tp[:, 128:256], och_pair[i1][:], ident_s[:])
                if p % 2 == 0:
                    nc.vector.tensor_copy(outT01[:, cpair], tp[:])
                else:
                    nc.scalar.activation(outT01[:, cpair], tp[:], COPY)
            elif h == 2:
                tp2 = psum.tile([D, 256], BF16, name="tp2", tag="A")
                nc.tensor.transpose(tp2[:, 0:128], och2[i0][:], ident_s[:])
                nc.tensor.transpose(tp2[:, 128:256], och2[i1][:], ident_s[:])
                if p % 2 == 0:
                    nc.scalar.activation(outT2[:, cpair], tp2[:], COPY)
                else:
                    nc.vector.tensor_copy(outT2[:, cpair], tp2[:])
                if p % 2 == 1:
                    emit_yproj(p // 2)


def _yproj_maker(nc, psum, ysb, wpt_a, wpt_b, outT01, outT2, yt):
    def emit_yproj(nh):
        cols = slice(nh * 512, (nh + 1) * 512)
        for ot in range(C // 128):
            ypt = psum.tile([128, 512], FP32, name="ypt", tag="A")
            nc.tensor.matmul(
                ypt[:],
                wpt_a[:, ot * 128 : (ot + 1) * 128],
                outT01[:, cols],
                start=True,
                stop=False,
            )
            nc.tensor.matmul(
                ypt[:],
                wpt_b[:, ot * 128 : (ot + 1) * 128],
                outT2[:, cols],
                start=False,
                stop=True,
            )
            yo = ysb.tile([128, 512], FP32, name="yo", tag="yo")
            if ot % 2 == 0:
                nc.vector.tensor_copy(yo[:], ypt[:])
            else:
                nc.scalar.activation(yo[:], ypt[:], COPY)
            nc.sync.dma_start(yt[ot * 128 : (ot + 1) * 128, cols], yo[:])

    return emit_yproj


_PROGRAM = None


def _get_program():
    global _PROGRAM
    if _PROGRAM is None:
        _PROGRAM = build_program()
    return _PROGRAM


def _bf16(a):
    return np.ascontiguousarray(a).astype(ml_dtypes.bfloat16)


def make_core_inputs(x, W_attn, W_proj, proj, core):
    b, hg = divmod(core, 4)
    heads = list(range(HPC * hg, HPC * (hg + 1)))
    rows = []
    for h in heads:  # (q_h | k_h) pairs, then the v block
        rows.extend(range(h * D, (h + 1) * D))
        rows.extend(range(C + h * D, C + (h + 1) * D))
    for h in heads:
        rows.extend(range(2 * C + h * D, 2 * C + (h + 1) * D))
    projext = np.concatenate(
        [proj.astype(np.float32), np.full((D, F), -0.5, np.float32)], axis=0
    )
    tri = np.triu(np.ones((S, S), np.float32))
    mask3 = np.concatenate([tri, np.ones((S, S), np.float32), tri], axis=1)
    return {
        "xt": _bf16(x[b].T),
        "wt": _bf16(W_attn[rows, :].T),
        "wpt": _bf16(W_proj[:, CP * hg : CP * (hg + 1)].T),
        "projext": _bf16(projext),
        "mask3": _bf16(mask3),
        "ident": _bf16(np.eye(128, dtype=np.float32)),
    }


def kernel(x, W_attn, W_proj, proj):
    nc = _get_program()
    in_maps = [
        make_core_inputs(x, W_attn, W_proj, proj, core) for core in range(N_CORES)
    ]
    res = run_bass_kernel_spmd(nc, in_maps, list(range(N_CORES)))
    out = np.empty((B, T, C), np.float32)
    for b in range(B):
        acc = res.results[4 * b]["yt"].astype(np.float32).copy()
        for g in range(1, 4):
            acc += res.results[4 * b + g]["yt"]
        out[b] = acc.T
    return out
